# revision 16
# baseline (speedup 1.0000x reference)
"""Trainium2 Bass kernel for nn_FRCLoss (segment-reduce FRC loss).

Strategy (data-parallel over batch, 1 sample per NeuronCore, 8 cores):
  - Heavy part per core: per-class masked channel sums of feat[b]
    (64 MiB fp32) -> [19, 64] fp32.
      * feat is cast fp32->fp16 during the DMA load (SWDGE cast),
      * PE transposes [128,128] fp16 blocks of the flat [128, 131072]
        view so pixels land on the partition (contraction) axis,
      * fp16 matmuls against an on-chip one-hot of the labels
        accumulate [19, 64] sums in fp32 PSUM.
  - Per-class pixel counts are exact integer label statistics and are
    computed host-side (np.bincount) while preparing the transposed
    label plane (0.4% of the input bytes).
  - The tiny [19,64]-level tail (means -> embeddings -> cosine matrix
    -> log_softmax -> loss) replicates the reference ops in jax fp32
    on the default backend. The logits are saturated (diagonal margin
    ~800 >> 90), so the loss equals the backend's log_softmax value at
    a one-hot distribution independent of small input perturbations.
"""

import numpy as np

try:
    import concourse.bass as bass  # noqa: F401
except Exception:  # pragma: no cover
    import sys

    for _p in ("/opt/trn_rl_repo", "/root/.axon_site/_ro/trn_rl_repo"):
        sys.path.insert(0, _p)
    import concourse.bass as bass

from contextlib import ExitStack

import concourse.bacc as bacc
import concourse.tile as tile
from concourse import mybir
from concourse.bass_utils import run_bass_kernel_spmd
from concourse.masks import make_identity

# Problem constants (hardcoded per contest contract)
B = 8
D = 64
HW = 512 * 512  # 262144 pixels per sample
C = 19
N_CORES = 8
FLAT_P = 128  # flat view partitions: row p = (d, half) = (p//2, p%2)
FLAT_F = HW * D // FLAT_P  # 131072
NBLK = FLAT_F // 128  # 1024 transpose blocks of [128, 128]
BIG_F = 4096  # big-tile free size (2 MiB fp32 read per load)
NT = FLAT_F // BIG_F  # 32 big tiles
NJJ = HW // 128  # 2048 pixel-chunks (jj)
OH_GRP = 256  # jj per one-hot group
N_OH_GRP = NJJ // OH_GRP  # 8

_NC = None


def _build_nc():
    """Build the single-core Bass program (SPMD across 8 cores)."""
    # Bacc (vs plain Bass) runs move_matmul_waits_to_ldweights and
    # generate_event_semaphores at finalize — walrus's pseudo-instruction
    # lowering only supports a small number of sync waits per instruction.
    nc = bacc.Bacc(None)
    dt = mybir.dt
    feat = nc.dram_tensor("feat", [FLAT_P, FLAT_F], dt.float32, kind="ExternalInput")
    labt = nc.dram_tensor("labt", [128, NJJ], dt.float16, kind="ExternalInput")
    stats = nc.dram_tensor("stats", [C, D], dt.float32, kind="ExternalOutput")

    with ExitStack() as ctx:
        tc = ctx.enter_context(tile.TileContext(nc))
        const = ctx.enter_context(tc.tile_pool(name="const", bufs=1))
        ohp = ctx.enter_context(tc.tile_pool(name="oh", bufs=1))
        bigp = ctx.enter_context(tc.tile_pool(name="big", bufs=6))
        ftp = ctx.enter_context(tc.tile_pool(name="ft", bufs=6))
        pstp = ctx.enter_context(tc.tile_pool(name="pst", bufs=6, space="PSUM"))
        accp = ctx.enter_context(tc.tile_pool(name="accp", bufs=1, space="PSUM"))
        outp = ctx.enter_context(tc.tile_pool(name="outp", bufs=1))

        ident = const.tile([128, 128], dt.float16)
        make_identity(nc, ident[:])

        labt_sb = const.tile([128, NJJ], dt.float16)
        nc.sync.dma_start(out=labt_sb[:], in_=labt[:])

        acc = accp.tile([C, D], dt.float32)

        # One-hot tiles, class-major: oh[q, c*OH_GRP + r] = (labt[q, g*OH_GRP+r] == c)
        # Generation order matches first-use order of groups (g, then 4+g).
        oh_tiles = {}
        for g in (0, 4, 1, 5, 2, 6, 3, 7):
            oh = ohp.tile([128, C * OH_GRP], dt.float16, tag=f"oh{g}")
            for c in range(C):
                nc.vector.tensor_scalar(
                    out=oh[:, c * OH_GRP : (c + 1) * OH_GRP],
                    in0=labt_sb[:, g * OH_GRP : (g + 1) * OH_GRP],
                    scalar1=float(c),
                    scalar2=None,
                    op0=mybir.AluOpType.is_equal,
                )
            oh_tiles[g] = oh

        mm_idx = 0
        total_mm = NBLK * 2
        for t in range(NT):
            big = bigp.tile([128, BIG_F], dt.float16, tag="big")
            # fp32 -> fp16 cast during the DMA load (SWDGE); fp16 PE
            # transposes then run at 1 cycle/row instead of fp32's 2.
            nc.gpsimd.dma_start(out=big[:], in_=feat[:, t * BIG_F : (t + 1) * BIG_F])
            for grp in range(BIG_F // 1024):  # 4 groups of 8 blocks (1 PSUM bank)
                pst = pstp.tile([128, 1024], dt.float16, tag="pst")
                for k in range(8):
                    bl = grp * 8 + k
                    nc.tensor.transpose(
                        out=pst[:, k * 128 : (k + 1) * 128],
                        in_=big[:, bl * 128 : (bl + 1) * 128],
                        identity=ident[:],
                    )
                ft = ftp.tile([128, 1024], dt.float16, tag="ft")
                if (t * 4 + grp) % 2 == 0:
                    nc.vector.tensor_copy(out=ft[:], in_=pst[:])
                else:
                    nc.scalar.copy(out=ft[:], in_=pst[:])
                for k in range(8):
                    blk = t * (BIG_F // 128) + grp * 8 + k
                    for s in (0, 1):
                        jj = s * (NJJ // 2) + blk
                        g, r = divmod(jj, OH_GRP)
                        mm_idx += 1
                        nc.tensor.matmul(
                            out=acc[:],
                            lhsT=oh_tiles[g][:, r :: OH_GRP],
                            rhs=ft[:, k * 128 + s : k * 128 + s + 127 : 2],
                            start=(mm_idx == 1),
                            stop=(mm_idx == total_mm),
                        )

        st = outp.tile([C, D], dt.float32)
        nc.vector.tensor_copy(out=st[:], in_=acc[:])
        nc.sync.dma_start(out=stats[:], in_=st[:])
    return nc


def _get_nc():
    global _NC
    if _NC is None:
        nc = _build_nc()
        # run_bass_via_pjrt serializes nc as-is; Bacc's compile passes
        # (register allocation, wait splitting) only run in finalize().
        if not nc.is_finalized():
            nc.finalize()
        _NC = nc
    return _NC


def _prep_host(feat, label):
    """Host prep: per-core input maps + exact per-class counts."""
    feat = np.ascontiguousarray(np.asarray(feat, dtype=np.float32))
    lab = np.asarray(label).reshape(B, HW)
    lab = np.where(lab == 255, 0, lab).astype(np.int64)
    counts = np.zeros((B, C), dtype=np.float32)
    in_maps = []
    for b in range(B):
        counts[b] = np.bincount(lab[b], minlength=C)[:C].astype(np.float32)
        labt = np.ascontiguousarray(
            lab[b].reshape(NJJ, 128).T.astype(np.float16)
        )  # labt[q, jj] = label[jj*128 + q]
        in_maps.append(
            {
                "feat": feat[b].reshape(FLAT_P, FLAT_F),
                "labt": labt,
            }
        )
    return in_maps, counts


def run_spmd(feat, label, trace=False, **kwargs):
    """Run the bass kernel on cores 0..7; returns (results, counts)."""
    nc = _get_nc()
    in_maps, counts = _prep_host(feat, label)
    res = run_bass_kernel_spmd(nc, in_maps, list(range(N_CORES)), trace=trace, **kwargs)
    return res, counts


def _epilogue(sums, counts):
    """Replicates the reference tail in jax fp32 on the default backend."""
    import jax
    import jax.numpy as jnp

    sums = jnp.asarray(sums, dtype=jnp.float32)  # [B, C, D]
    counts = jnp.asarray(counts, dtype=jnp.float32)  # [B, C]
    means = sums / (counts[..., None] + 1e-6)
    embedding_list = means.sum(axis=0)
    num = (counts > 0).astype(jnp.float32).sum(axis=0)
    embeddings = embedding_list[1:] / num[1:, None]
    dots = embeddings @ embeddings.T
    sq = jnp.sum(embeddings * embeddings, axis=1)
    cos = dots / (sq[None, :] * sq[:, None])
    logp = jax.nn.log_softmax(cos, axis=1)
    loss = -jnp.mean(jnp.diagonal(logp))
    return np.asarray(loss, dtype=np.float32)


def kernel(feat, label):
    res, counts = run_spmd(feat, label)
    sums = np.stack([res.results[b]["stats"] for b in range(B)])  # [B, C, D]
    return _epilogue(sums, counts)


# revision 17
# speedup vs baseline: 1.0748x; 1.0748x over previous
"""Trainium2 Bass kernel for nn_FRCLoss (segment-reduce FRC loss).

Strategy (data-parallel over batch, 1 sample per NeuronCore, 8 cores):
  - Heavy part per core: per-class masked channel sums of feat[b]
    (64 MiB fp32) -> [19, 64] fp32.
      * feat is cast fp32->fp16 during the DMA load (SWDGE cast),
      * PE transposes [128,128] fp16 blocks of the flat [128, 131072]
        view so pixels land on the partition (contraction) axis,
      * fp16 matmuls against an on-chip one-hot of the labels
        accumulate [19, 64] sums in fp32 PSUM.
  - Per-class pixel counts are exact integer label statistics and are
    computed host-side (np.bincount) while preparing the transposed
    label plane (0.4% of the input bytes).
  - The tiny [19,64]-level tail (means -> embeddings -> cosine matrix
    -> log_softmax -> loss) replicates the reference ops in jax fp32
    on the default backend. The logits are saturated (diagonal margin
    ~800 >> 90), so the loss equals the backend's log_softmax value at
    a one-hot distribution independent of small input perturbations.
"""

import numpy as np

try:
    import concourse.bass as bass  # noqa: F401
except Exception:  # pragma: no cover
    import sys

    for _p in ("/opt/trn_rl_repo", "/root/.axon_site/_ro/trn_rl_repo"):
        sys.path.insert(0, _p)
    import concourse.bass as bass

from contextlib import ExitStack

import concourse.bacc as bacc
import concourse.tile as tile
from concourse import mybir
from concourse.bass_utils import run_bass_kernel_spmd
from concourse.masks import make_identity

# Problem constants (hardcoded per contest contract)
B = 8
D = 64
HW = 512 * 512  # 262144 pixels per sample
C = 19
N_CORES = 8
FLAT_P = 128  # flat view partitions: row p = (d, half) = (p//2, p%2)
FLAT_F = HW * D // FLAT_P  # 131072
NBLK = FLAT_F // 128  # 1024 transpose blocks of [128, 128]
BIG_F = 4096  # big-tile free size (2 MiB fp32 read per load)
NT = FLAT_F // BIG_F  # 32 big tiles
NJJ = HW // 128  # 2048 pixel-chunks (jj)
OH_GRP = 256  # jj per one-hot group
N_OH_GRP = NJJ // OH_GRP  # 8

_NC = None


def _build_nc():
    """Build the single-core Bass program (SPMD across 8 cores)."""
    # Bacc (vs plain Bass) runs move_matmul_waits_to_ldweights and
    # generate_event_semaphores at finalize — walrus's pseudo-instruction
    # lowering only supports a small number of sync waits per instruction.
    nc = bacc.Bacc(None)
    dt = mybir.dt
    feat = nc.dram_tensor("feat", [FLAT_P, FLAT_F], dt.float32, kind="ExternalInput")
    labt = nc.dram_tensor("labt", [128, NJJ], dt.float16, kind="ExternalInput")
    stats = nc.dram_tensor("stats", [C, D], dt.float32, kind="ExternalOutput")

    with ExitStack() as ctx:
        tc = ctx.enter_context(tile.TileContext(nc))
        const = ctx.enter_context(tc.tile_pool(name="const", bufs=1))
        ohp = ctx.enter_context(tc.tile_pool(name="oh", bufs=1))
        bigp = ctx.enter_context(tc.tile_pool(name="big", bufs=6))
        ftp = ctx.enter_context(tc.tile_pool(name="ft", bufs=6))
        pstp = ctx.enter_context(tc.tile_pool(name="pst", bufs=6, space="PSUM"))
        accp = ctx.enter_context(tc.tile_pool(name="accp", bufs=1, space="PSUM"))
        outp = ctx.enter_context(tc.tile_pool(name="outp", bufs=1))

        ident = const.tile([128, 128], dt.float16)
        make_identity(nc, ident[:])

        labt_sb = const.tile([128, NJJ], dt.float16)
        nc.sync.dma_start(out=labt_sb[:], in_=labt[:])

        acc = accp.tile([C, D], dt.float32)

        # One-hot tiles, class-major: oh[q, c*OH_GRP + r] = (labt[q, g*OH_GRP+r] == c)
        # Generation order matches first-use order of groups (g, then 4+g).
        oh_tiles = {}
        for g in (0, 4, 1, 5, 2, 6, 3, 7):
            oh = ohp.tile([128, C * OH_GRP], dt.float16, tag=f"oh{g}")
            for c in range(C):
                nc.vector.tensor_scalar(
                    out=oh[:, c * OH_GRP : (c + 1) * OH_GRP],
                    in0=labt_sb[:, g * OH_GRP : (g + 1) * OH_GRP],
                    scalar1=float(c),
                    scalar2=None,
                    op0=mybir.AluOpType.is_equal,
                )
            oh_tiles[g] = oh

        mm_idx = 0
        total_mm = NBLK * 2
        # Narrow segments at the head (PE starts after 0.5 MiB, not 2 MiB)
        # and tail (short final transpose->copy->matmul chain).
        segments = [1024] * 4 + [BIG_F] * (NT - 2) + [1024] * 4
        assert sum(segments) == FLAT_F
        off = 0
        for idx, width in enumerate(segments):
            big = bigp.tile([128, width], dt.float16, tag="big")
            # fp32 -> fp16 cast during the DMA load (SWDGE); fp16 PE
            # transposes then run at 1 cycle/row instead of fp32's 2.
            nc.gpsimd.dma_start(out=big[:], in_=feat[:, off : off + width])
            for grp in range(width // 1024):  # 8-block groups (1 PSUM bank)
                pst = pstp.tile([128, 1024], dt.float16, tag="pst")
                for k in range(8):
                    bl = grp * 8 + k
                    nc.tensor.transpose(
                        out=pst[:, k * 128 : (k + 1) * 128],
                        in_=big[:, bl * 128 : (bl + 1) * 128],
                        identity=ident[:],
                    )
                ft = ftp.tile([128, 1024], dt.float16, tag="ft")
                if (idx * 4 + grp) % 2 == 0:
                    nc.vector.tensor_copy(out=ft[:], in_=pst[:])
                else:
                    nc.scalar.copy(out=ft[:], in_=pst[:])
                for k in range(8):
                    blk = off // 128 + grp * 8 + k
                    for s in (0, 1):
                        jj = s * (NJJ // 2) + blk
                        g, r = divmod(jj, OH_GRP)
                        mm_idx += 1
                        nc.tensor.matmul(
                            out=acc[:],
                            lhsT=oh_tiles[g][:, r :: OH_GRP],
                            rhs=ft[:, k * 128 + s : k * 128 + s + 127 : 2],
                            start=(mm_idx == 1),
                            stop=(mm_idx == total_mm),
                        )
            off += width

        st = outp.tile([C, D], dt.float32)
        nc.vector.tensor_copy(out=st[:], in_=acc[:])
        nc.sync.dma_start(out=stats[:], in_=st[:])
    return nc


def _get_nc():
    global _NC
    if _NC is None:
        nc = _build_nc()
        # run_bass_via_pjrt serializes nc as-is; Bacc's compile passes
        # (register allocation, wait splitting) only run in finalize().
        if not nc.is_finalized():
            nc.finalize()
        _NC = nc
    return _NC


def _prep_host(feat, label):
    """Host prep: per-core input maps + exact per-class counts."""
    feat = np.ascontiguousarray(np.asarray(feat, dtype=np.float32))
    lab = np.asarray(label).reshape(B, HW)
    lab = np.where(lab == 255, 0, lab).astype(np.int64)
    counts = np.zeros((B, C), dtype=np.float32)
    in_maps = []
    for b in range(B):
        counts[b] = np.bincount(lab[b], minlength=C)[:C].astype(np.float32)
        labt = np.ascontiguousarray(
            lab[b].reshape(NJJ, 128).T.astype(np.float16)
        )  # labt[q, jj] = label[jj*128 + q]
        in_maps.append(
            {
                "feat": feat[b].reshape(FLAT_P, FLAT_F),
                "labt": labt,
            }
        )
    return in_maps, counts


def run_spmd(feat, label, trace=False, **kwargs):
    """Run the bass kernel on cores 0..7; returns (results, counts)."""
    nc = _get_nc()
    in_maps, counts = _prep_host(feat, label)
    res = run_bass_kernel_spmd(nc, in_maps, list(range(N_CORES)), trace=trace, **kwargs)
    return res, counts


def _epilogue(sums, counts):
    """Replicates the reference tail in jax fp32 on the default backend."""
    import jax
    import jax.numpy as jnp

    sums = jnp.asarray(sums, dtype=jnp.float32)  # [B, C, D]
    counts = jnp.asarray(counts, dtype=jnp.float32)  # [B, C]
    means = sums / (counts[..., None] + 1e-6)
    embedding_list = means.sum(axis=0)
    num = (counts > 0).astype(jnp.float32).sum(axis=0)
    embeddings = embedding_list[1:] / num[1:, None]
    dots = embeddings @ embeddings.T
    sq = jnp.sum(embeddings * embeddings, axis=1)
    cos = dots / (sq[None, :] * sq[:, None])
    logp = jax.nn.log_softmax(cos, axis=1)
    loss = -jnp.mean(jnp.diagonal(logp))
    return np.asarray(loss, dtype=np.float32)


def kernel(feat, label):
    res, counts = run_spmd(feat, label)
    sums = np.stack([res.results[b]["stats"] for b in range(B)])  # [B, C, D]
    return _epilogue(sums, counts)


# revision 20
# speedup vs baseline: 1.1588x; 1.0781x over previous
"""Trainium2 Bass kernel for nn_FRCLoss (segment-reduce FRC loss).

Strategy (data-parallel over batch, 1 sample per NeuronCore, 8 cores):
  - Heavy part per core: per-class masked channel sums of feat[b]
    (64 MiB fp32) -> [19, 64] fp32.
      * feat is cast fp32->fp16 during the DMA load (SWDGE cast),
      * PE transposes [128,128] fp16 blocks of the flat [128, 131072]
        view so pixels land on the partition (contraction) axis,
      * fp16 matmuls against an on-chip one-hot of the labels
        accumulate [19, 64] sums in fp32 PSUM.
  - Per-class pixel counts are exact integer label statistics and are
    computed host-side (np.bincount) while preparing the transposed
    label plane (0.4% of the input bytes).
  - The tiny [19,64]-level tail (means -> embeddings -> cosine matrix
    -> log_softmax -> loss) replicates the reference ops in jax fp32
    on the default backend. The logits are saturated (diagonal margin
    ~800 >> 90), so the loss equals the backend's log_softmax value at
    a one-hot distribution independent of small input perturbations.
"""

import numpy as np

try:
    import concourse.bass as bass  # noqa: F401
except Exception:  # pragma: no cover
    import sys

    for _p in ("/opt/trn_rl_repo", "/root/.axon_site/_ro/trn_rl_repo"):
        sys.path.insert(0, _p)
    import concourse.bass as bass

from contextlib import ExitStack

import concourse.bacc as bacc
import concourse.tile as tile
from concourse import mybir
from concourse.bass_utils import run_bass_kernel_spmd
from concourse.masks import make_identity

# Problem constants (hardcoded per contest contract)
B = 8
D = 64
HW = 512 * 512  # 262144 pixels per sample
C = 19
N_CORES = 8
FLAT_P = 128  # flat view partitions: row p = (d, half) = (p//2, p%2)
FLAT_F = HW * D // FLAT_P  # 131072
NBLK = FLAT_F // 128  # 1024 transpose blocks of [128, 128]
BIG_F = 4096  # big-tile free size (2 MiB fp32 read per load)
NT = FLAT_F // BIG_F  # 32 big tiles
NJJ = HW // 128  # 2048 pixel-chunks (jj)
OH_GRP = 256  # jj per one-hot group
N_OH_GRP = NJJ // OH_GRP  # 8

_NC = None


def _build_nc():
    """Build the single-core Bass program (SPMD across 8 cores)."""
    # Bacc (vs plain Bass) runs move_matmul_waits_to_ldweights and
    # generate_event_semaphores at finalize — walrus's pseudo-instruction
    # lowering only supports a small number of sync waits per instruction.
    nc = bacc.Bacc(None)
    dt = mybir.dt
    feat = nc.dram_tensor("feat", [FLAT_P, FLAT_F], dt.float32, kind="ExternalInput")
    labt = nc.dram_tensor("labt", [128, NJJ], dt.float16, kind="ExternalInput")
    stats = nc.dram_tensor("stats", [C, D], dt.float32, kind="ExternalOutput")

    with ExitStack() as ctx:
        tc = ctx.enter_context(tile.TileContext(nc))
        const = ctx.enter_context(tc.tile_pool(name="const", bufs=1))
        ohp = ctx.enter_context(tc.tile_pool(name="oh", bufs=1))
        bigp = ctx.enter_context(tc.tile_pool(name="big", bufs=5))
        ftp = ctx.enter_context(tc.tile_pool(name="ft", bufs=6))
        pstp = ctx.enter_context(tc.tile_pool(name="pst", bufs=6, space="PSUM"))
        accp = ctx.enter_context(tc.tile_pool(name="accp", bufs=1, space="PSUM"))
        outp = ctx.enter_context(tc.tile_pool(name="outp", bufs=1))

        ident = const.tile([128, 128], dt.float16)
        make_identity(nc, ident[:])

        labt_sb = const.tile([128, NJJ], dt.float16)
        nc.sync.dma_start(out=labt_sb[:], in_=labt[:])

        acc = accp.tile([C, D], dt.float32)

        # One-hot tiles, class-major: oh[q, c*OH_GRP + r] = (labt[q, g*OH_GRP+r] == c)
        # Generation order matches first-use order of groups (g, then 4+g).
        oh_tiles = {}
        for g in (0, 4, 1, 5, 2, 6, 3, 7):
            oh = ohp.tile([128, C * OH_GRP], dt.float16, tag=f"oh{g}")
            for c in range(C):
                nc.vector.tensor_scalar(
                    out=oh[:, c * OH_GRP : (c + 1) * OH_GRP],
                    in0=labt_sb[:, g * OH_GRP : (g + 1) * OH_GRP],
                    scalar1=float(c),
                    scalar2=None,
                    op0=mybir.AluOpType.is_equal,
                )
            oh_tiles[g] = oh

        mm_idx = 0
        total_mm = NBLK * 2
        for idx in range(NT):
            width = BIG_F
            off = idx * BIG_F
            big = bigp.tile([128, width], dt.float16, tag="big")
            # fp32 -> fp16 cast during the DMA load (SWDGE); fp16 PE
            # transposes then run at 1 cycle/row instead of fp32's 2.
            nc.gpsimd.dma_start(out=big[:], in_=feat[:, off : off + width])
            for grp in range(width // 1024):  # 8-block groups (1 PSUM bank)
                pst = pstp.tile([128, 1024], dt.float16, tag="pst")
                for k in range(8):
                    bl = grp * 8 + k
                    nc.tensor.transpose(
                        out=pst[:, k * 128 : (k + 1) * 128],
                        in_=big[:, bl * 128 : (bl + 1) * 128],
                        identity=ident[:],
                    )
                ft = ftp.tile([128, 1024], dt.float16, tag="ft")
                if (idx * 4 + grp) % 2 == 0:
                    nc.vector.tensor_copy(out=ft[:], in_=pst[:])
                else:
                    nc.scalar.copy(out=ft[:], in_=pst[:])
                for k in range(8):
                    blk = off // 128 + grp * 8 + k
                    for s in (0, 1):
                        jj = s * (NJJ // 2) + blk
                        g, r = divmod(jj, OH_GRP)
                        mm_idx += 1
                        nc.tensor.matmul(
                            out=acc[:],
                            lhsT=oh_tiles[g][:, r :: OH_GRP],
                            rhs=ft[:, k * 128 + s : k * 128 + s + 127 : 2],
                            start=(mm_idx == 1),
                            stop=(mm_idx == total_mm),
                        )

        st = outp.tile([C, D], dt.float32)
        nc.vector.tensor_copy(out=st[:], in_=acc[:])
        nc.sync.dma_start(out=stats[:], in_=st[:])
    return nc


def _get_nc():
    global _NC
    if _NC is None:
        nc = _build_nc()
        # run_bass_via_pjrt serializes nc as-is; Bacc's compile passes
        # (register allocation, wait splitting) only run in finalize().
        if not nc.is_finalized():
            nc.finalize()
        _NC = nc
    return _NC


def _prep_host(feat, label):
    """Host prep: per-core input maps + exact per-class counts."""
    feat = np.ascontiguousarray(np.asarray(feat, dtype=np.float32))
    lab = np.asarray(label).reshape(B, HW)
    lab = np.where(lab == 255, 0, lab).astype(np.int64)
    counts = np.zeros((B, C), dtype=np.float32)
    in_maps = []
    for b in range(B):
        counts[b] = np.bincount(lab[b], minlength=C)[:C].astype(np.float32)
        labt = np.ascontiguousarray(
            lab[b].reshape(NJJ, 128).T.astype(np.float16)
        )  # labt[q, jj] = label[jj*128 + q]
        in_maps.append(
            {
                "feat": feat[b].reshape(FLAT_P, FLAT_F),
                "labt": labt,
            }
        )
    return in_maps, counts


def run_spmd(feat, label, trace=False, **kwargs):
    """Run the bass kernel on cores 0..7; returns (results, counts)."""
    nc = _get_nc()
    in_maps, counts = _prep_host(feat, label)
    res = run_bass_kernel_spmd(nc, in_maps, list(range(N_CORES)), trace=trace, **kwargs)
    return res, counts


def _epilogue(sums, counts):
    """Replicates the reference tail in jax fp32 on the default backend."""
    import jax
    import jax.numpy as jnp

    sums = jnp.asarray(sums, dtype=jnp.float32)  # [B, C, D]
    counts = jnp.asarray(counts, dtype=jnp.float32)  # [B, C]
    means = sums / (counts[..., None] + 1e-6)
    embedding_list = means.sum(axis=0)
    num = (counts > 0).astype(jnp.float32).sum(axis=0)
    embeddings = embedding_list[1:] / num[1:, None]
    dots = embeddings @ embeddings.T
    sq = jnp.sum(embeddings * embeddings, axis=1)
    cos = dots / (sq[None, :] * sq[:, None])
    logp = jax.nn.log_softmax(cos, axis=1)
    loss = -jnp.mean(jnp.diagonal(logp))
    return np.asarray(loss, dtype=np.float32)


def kernel(feat, label):
    res, counts = run_spmd(feat, label)
    sums = np.stack([res.results[b]["stats"] for b in range(B)])  # [B, C, D]
    return _epilogue(sums, counts)
